# revision 1
# baseline (speedup 1.0000x reference)
"""Llama attention (B=2, S=2048, E=4096, H=32) on 8 trn2 NeuronCores.

Strategy (tensor-parallel over heads, 4 heads/core):
  - RoPE here is position-independent (cos/sin are [H, D/2], broadcast over
    seq), so it is a fixed per-head linear rotation folded into wq/wk on the
    host.  The 1/sqrt(D) score scale is folded into wq as well.
  - Scores are computed transposed (S^T = K^T-tile @ Q^T) and the attention
    output as O^T = V-tile @ P^T, so the device kernel is pure matmuls +
    exp with zero on-device transposes.  Softmax needs no max-subtraction
    (scores are bounded ~ +-8 here; fp32 exp cannot overflow).
  - Softmax denominators via an M=1 ones-matmul on the PE; the reciprocal is
    broadcast across partitions with a K=1 ones-matmul.
  - All matmuls run as float32r (full fp32 data, 1 cycle/row for N>=256).
  - Per-core output is a partial Y (row-sharded wo); host sums the 8 partials.
"""

import sys

sys.path.insert(0, "/opt/trn_rl_repo")

import numpy as np

B, S, E, H = 2, 2048, 4096, 32
D = 128            # head dim
NCORES = 8
HL = H // NCORES   # heads per core = 4
W = HL * D         # per-core projection width = 512
T = B * S          # 4096 tokens
KB = 8             # contraction blocks over E (512 each)
KK = 4             # 128-row k-tiles per block
NCH = 4            # 512-token chunks per batch
CH = 512

_CACHE = {}


def _build_nc():
    import concourse.bass as bass  # noqa: F401
    import concourse.mybir as mybir
    import concourse.tile as tile
    from concourse import bacc

    fp32 = mybir.dt.float32
    fp32r = mybir.dt.float32r
    EXP = mybir.ActivationFunctionType.Exp

    nc = bacc.Bacc("TRN2", target_bir_lowering=False, debug=False)

    xT_d = nc.dram_tensor("xT", [E, T], fp32r, kind="ExternalInput")
    wqk_d = nc.dram_tensor("wqk", [KB, 128, KK, 2 * W], fp32r, kind="ExternalInput")
    wv_d = nc.dram_tensor("wv", [KB, 128, KK, W], fp32r, kind="ExternalInput")
    wo_d = nc.dram_tensor("wo", [128, KK, E], fp32r, kind="ExternalInput")
    y_d = nc.dram_tensor("y", [T, E], fp32, kind="ExternalOutput")

    xview = xT_d.rearrange("(kb kk p) t -> kb p kk t", kk=KK, p=128)

    from concourse.bass_isa import ReduceOp

    with nc.allow_low_precision(reason="fp32r feeds PE; rounding is intended"), \
         tile.TileContext(nc) as tc:
        with tc.tile_pool(name="const", bufs=1) as constp, \
             tc.tile_pool(name="g_wo", bufs=2) as wop, \
             tc.tile_pool(name="g_psA", bufs=6, space="PSUM") as psA, \
             tc.tile_pool(name="g_psB", bufs=2, space="PSUM") as psB:
            zbias = constp.tile([128, 1], fp32, tag="zbias")
            nc.vector.memset(zbias[:], 0.0)

            for b in range(B):
                with tc.tile_pool(name=f"ot{b}", bufs=1) as otp:
                    OT = [otp.tile([128, S], fp32r, tag=f"ot{i}", name=f"ot{i}") for i in range(HL)]
                    with tc.tile_pool(name=f"qkv{b}", bufs=1) as qkvp:
                        QT = [qkvp.tile([128, S], fp32r, tag=f"qt{i}", name=f"qt{i}") for i in range(HL)]
                        KT = [qkvp.tile([128, S], fp32r, tag=f"kt{i}", name=f"kt{i}") for i in range(HL)]
                        V = [qkvp.tile([128, W], fp32r, tag=f"v{i}", name=f"v{i}") for i in range(4 * NCH)]

                        # ---------------- phase 1: projections ----------------
                        with tc.tile_pool(name=f"p1w{b}", bufs=2) as wpool, \
                             tc.tile_pool(name=f"p1wv{b}", bufs=1) as wvpool, \
                             tc.tile_pool(name=f"p1x{b}", bufs=2) as xpool:
                            for kb in range(KB):
                                wqk_t = wpool.tile([128, KK, 2 * W], fp32r, tag="wqk")
                                nc.sync.dma_start(wqk_t[:], wqk_d[kb])
                                wv_t = wvpool.tile([128, KK, W], fp32r, tag="wv")
                                nc.sync.dma_start(wv_t[:], wv_d[kb])
                                for n in range(NCH):
                                    tok0 = b * S + n * CH
                                    xc = xpool.tile([128, KK, CH], fp32r, tag="xc")
                                    nc.sync.dma_start(
                                        xc[:], xview[kb, :, :, tok0:tok0 + CH]
                                    )
                                    for proj in range(2):  # 0 -> QT, 1 -> KT
                                        for mi in range(HL):
                                            ps = psA.tile([128, CH], fp32, tag="psA", name="ps")
                                            c0 = proj * W + mi * 128
                                            for kk in range(KK):
                                                nc.tensor.matmul(
                                                    ps[:],
                                                    wqk_t[:, kk, c0:c0 + 128],
                                                    xc[:, kk, :],
                                                    start=(kk == 0),
                                                    stop=(kk == KK - 1),
                                                )
                                            dst = (QT if proj == 0 else KT)[mi][:, n * CH:(n + 1) * CH]
                                            if kb == 0:
                                                nc.vector.tensor_copy(dst, ps[:])
                                            else:
                                                nc.vector.tensor_add(dst, dst, ps[:])
                                    for mt in range(4):  # V: token tiles in chunk
                                        ps = psA.tile([128, W], fp32, tag="psA", name="psv")
                                        for kk in range(KK):
                                            nc.tensor.matmul(
                                                ps[:],
                                                xc[:, kk, mt * 128:(mt + 1) * 128],
                                                wv_t[:, kk, :],
                                                start=(kk == 0),
                                                stop=(kk == KK - 1),
                                            )
                                        vt = V[n * 4 + mt]
                                        if kb == 0:
                                            nc.vector.tensor_copy(vt[:], ps[:])
                                        else:
                                            nc.vector.tensor_add(vt[:], vt[:], ps[:])

                        # ---------------- phase 2: attention ----------------
                        # Software-pipelined by one (h, sq) chunk: chunk i's
                        # S^T matmuls + exps interleave with chunk i-1's
                        # V-accumulation matmuls, so the po chain never waits
                        # on a fresh exp and the PE never stalls on ACT.
                        with tc.tile_pool(name=f"a2e{b}", bufs=18) as ep, \
                             tc.tile_pool(name=f"a2s{b}", bufs=4) as esp, \
                             tc.tile_pool(name=f"a2r{b}", bufs=2) as rcp, \
                             tc.tile_pool(name=f"a2y{b}", bufs=3) as yp:
                            chunks = [(h, sq) for sq in range(4) for h in range(HL)]
                            state = {}  # chunk -> (eS list, po, esumA, esumB)

                            def emit_consume(ci, sk):
                                h, sq = ci
                                eS_l, po, esA, esB = state[ci]
                                nc.tensor.matmul(
                                    po[:],
                                    V[sk][:, h * 128:(h + 1) * 128],
                                    eS_l[sk][:],
                                    start=(sk == 0), stop=(sk == 15),
                                )
                                dst = esA if sk % 2 == 0 else esB
                                if sk < 2:
                                    nc.vector.tensor_copy(dst[:], eS_l[sk][:])
                                else:
                                    nc.vector.tensor_add(dst[:], dst[:], eS_l[sk][:])

                            def emit_tail(ci):
                                h, sq = ci
                                q0 = sq * 512
                                _, po, esA, esB = state.pop(ci)
                                nc.vector.tensor_copy(OT[h][:, q0:q0 + 512], po[:])
                                nc.vector.tensor_add(esA[:], esA[:], esB[:])
                                nc.gpsimd.partition_all_reduce(
                                    esA[:], esA[:], 128, ReduceOp.add)
                                recip = rcp.tile([128, 512], fp32, tag="recip")
                                nc.vector.reciprocal(recip[:], esA[:])
                                nc.vector.tensor_mul(
                                    OT[h][:, q0:q0 + 512],
                                    OT[h][:, q0:q0 + 512], recip[:])

                            prev = None
                            for ci in chunks:
                                h, sq = ci
                                q0 = sq * 512
                                po = psB.tile([128, 512], fp32, tag="psB", name="po")
                                esumA = esp.tile([128, 512], fp32, tag="esumA")
                                esumB = esp.tile([128, 512], fp32, tag="esumB")
                                eS_l = []
                                state[ci] = (eS_l, po, esumA, esumB)
                                for sk in range(16):
                                    pS = psA.tile([128, 512], fp32, tag="psA", name="pS")
                                    nc.tensor.matmul(
                                        pS[:],
                                        KT[h][:, sk * 128:(sk + 1) * 128],
                                        QT[h][:, q0:q0 + 512],
                                        start=True, stop=True,
                                    )
                                    eS = ep.tile([128, 512], fp32r, tag="eS")
                                    nc.scalar.activation(eS[:], pS[:], EXP, bias=zbias[:, 0:1])
                                    eS_l.append(eS)
                                    if prev is not None:
                                        emit_consume(prev, sk)
                                if prev is not None:
                                    emit_tail(prev)
                                prev = ci
                            for sk in range(16):
                                emit_consume(prev, sk)
                            emit_tail(prev)

                    # ---------------- phase 3: output projection ----------------
                    with tc.tile_pool(name=f"p3y{b}", bufs=3) as yp3:
                        for nE in range(8):
                            wo_t = wop.tile([128, KK, 512], fp32r, tag="wo")
                            nc.sync.dma_start(
                                wo_t[:], wo_d[:, :, nE * 512:(nE + 1) * 512])
                            for m in range(16):
                                py = psB.tile([128, 512], fp32, tag="psB", name="py")
                                for kd in range(KK):
                                    nc.tensor.matmul(
                                        py[:],
                                        OT[kd][:, m * 128:(m + 1) * 128],
                                        wo_t[:, kd, :],
                                        start=(kd == 0), stop=(kd == KK - 1),
                                    )
                                yt = yp3.tile([128, 512], fp32, tag="yt")
                                nc.vector.tensor_copy(yt[:], py[:])
                                nc.sync.dma_start(
                                    y_d[b * S + m * 128: b * S + (m + 1) * 128,
                                        nE * 512:(nE + 1) * 512],
                                    yt[:],
                                )

    nc.compile()
    return nc


def _prep_inputs(x, freqs_cos, freqs_sin, wq, wk, wv, wo):
    x = np.asarray(x, np.float32)
    c = np.asarray(freqs_cos, np.float32)
    s = np.asarray(freqs_sin, np.float32)
    wq = np.asarray(wq, np.float32)
    wk = np.asarray(wk, np.float32)
    wv = np.asarray(wv, np.float32)
    wo = np.asarray(wo, np.float32)

    xT = np.ascontiguousarray(x.reshape(T, E).T)

    def fold(w):
        wr = w.reshape(H, D // 2, 2, E)
        w0, w1 = wr[:, :, 0], wr[:, :, 1]
        r0 = c[:, :, None] * w0 - s[:, :, None] * w1
        r1 = s[:, :, None] * w0 + c[:, :, None] * w1
        return np.stack([r0, r1], axis=2).reshape(E, E)

    wq_r = fold(wq) * np.float32(D ** -0.5)
    wk_r = fold(wk)

    in_maps = []
    for cix in range(NCORES):
        sl = slice(cix * W, (cix + 1) * W)
        wqT = wq_r[sl].T                      # [E, W]
        wkT = wk_r[sl].T
        qk = np.concatenate([wqT, wkT], axis=1)          # [E, 2W]
        wqk = np.ascontiguousarray(
            qk.reshape(KB, KK, 128, 2 * W).transpose(0, 2, 1, 3))
        wvb = np.ascontiguousarray(
            wv[sl].T.reshape(KB, KK, 128, W).transpose(0, 2, 1, 3))
        wob = np.ascontiguousarray(
            wo[:, sl].T.reshape(KK, 128, E).transpose(1, 0, 2))
        in_maps.append({"xT": xT, "wqk": wqk, "wv": wvb, "wo": wob})
    return in_maps


def run(x, freqs_cos, freqs_sin, wq, wk, wv, wo, trace=False, tmpdir=None):
    from concourse.bass_utils import run_bass_kernel_spmd

    if "nc" not in _CACHE:
        _CACHE["nc"] = _build_nc()
    nc = _CACHE["nc"]
    in_maps = _prep_inputs(x, freqs_cos, freqs_sin, wq, wk, wv, wo)
    res = run_bass_kernel_spmd(
        nc, in_maps, list(range(NCORES)), trace=trace, tmpdir=tmpdir
    )
    y = res.results[0]["y"]
    for r in res.results[1:]:
        y = y + r["y"]
    return np.asarray(y, np.float32).reshape(B, S, E), res


def kernel(x, start_pos=0, freqs_cos=None, freqs_sin=None,
           wq=None, wk=None, wv=None, wo=None):
    y, _ = run(x, freqs_cos, freqs_sin, wq, wk, wv, wo)
    return y



# revision 3
# speedup vs baseline: 1.2816x; 1.2816x over previous
"""Llama attention (B=2, S=2048, E=4096, H=32) on 8 trn2 NeuronCores.

Strategy (tensor-parallel over heads, 4 heads/core, all-bf16 datapath):
  - RoPE is position-independent here (cos/sin are [H, D/2]), so it is folded
    into wq/wk on the host; the 1/sqrt(D) scale is folded into wq too.
  - All matmul operands are bf16 (1 cycle/row on the PE, fast weight load),
    accumulation in fp32 PSUM.  The full E=4096 contraction of the Q/K/V
    projections accumulates in PSUM (32 chained matmuls per output tile), so
    no vector-engine adds are needed; a single scalar-engine copy (with bf16
    cast) evacuates each tile.
  - Scores are computed transposed (S^T = K^T-tile @ Q^T) and the attention
    output as O^T = V-tile @ P^T: zero on-device transposes.  exp() runs on
    the scalar engine over [128,1024] PSUM spans.  Softmax denominators:
    bf16 DVE partial sums -> one ones-matmul (partition reduce on the PE) ->
    reciprocal_approx_fast -> K=1 ones-matmul broadcast -> one DVE multiply.
    Softmax needs no max-subtraction (scores bounded ~ +-8; exp can't
    overflow in fp32).
  - Per-core output is a partial Y (row-sharded wo) written bf16; the host
    sums the 8 partials in fp32.
"""

import sys

sys.path.insert(0, "/opt/trn_rl_repo")

import numpy as np
import ml_dtypes

B, S, E, H = 2, 2048, 4096, 32
D = 128            # head dim
NCORES = 8
HL = H // NCORES   # heads per core = 4
W = HL * D         # per-core projection width = 512
T = B * S          # 4096 tokens
NKB = 32           # 128-row contraction tiles over E
CH1 = 256          # phase-1 token chunk
NCH1 = S // CH1    # 8 chunks per batch
NTT = S // 128     # 16 token/key tiles per batch

_CACHE = {}


def _build_nc():
    import concourse.bass as bass  # noqa: F401
    import concourse.mybir as mybir
    import concourse.tile as tile
    from concourse import bacc

    fp32 = mybir.dt.float32
    bf16 = mybir.dt.bfloat16
    EXP = mybir.ActivationFunctionType.Exp

    nc = bacc.Bacc("TRN2", target_bir_lowering=False, debug=False)

    # host layouts (see _prep_inputs):
    #   xh   [T/CH1, 128, NKB, CH1]  x^T tiled per chunk
    #   wqkh [128, NKB, 2W]          [wq_r | wk_r] columns for this core
    #   wvh  [128, NKB, W]
    #   woh  [128, HL, E]
    xh_d = nc.dram_tensor("xh", [T // CH1, 128, NKB, CH1], bf16, kind="ExternalInput")
    wqk_d = nc.dram_tensor("wqk", [128, NKB, 2 * W], bf16, kind="ExternalInput")
    wv_d = nc.dram_tensor("wv", [128, NKB, W], bf16, kind="ExternalInput")
    wo_d = nc.dram_tensor("wo", [128, HL, E], bf16, kind="ExternalInput")
    y_d = nc.dram_tensor("y", [T, E], bf16, kind="ExternalOutput")

    with nc.allow_low_precision(reason="bf16 datapath; fp32 PSUM accumulation"), \
         tile.TileContext(nc) as tc:
        with tc.tile_pool(name="const", bufs=1) as constp, \
             tc.tile_pool(name="gw", bufs=1) as gwp:
            zbias = constp.tile([128, 1], fp32, tag="zbias")
            nc.vector.memset(zbias[:], 0.0)
            ones_col = constp.tile([128, 1], bf16, tag="ones_col")
            nc.vector.memset(ones_col[:], 1.0)
            ones_row = constp.tile([1, 128], fp32, tag="ones_row")
            nc.vector.memset(ones_row[:], 1.0)

            # resident weights: 8 MiB + 4 MiB
            wqk_t = gwp.tile([128, NKB, 2 * W], bf16, tag="wqk")
            nc.sync.dma_start(wqk_t[:], wqk_d[:])
            wv_t = gwp.tile([128, NKB, W], bf16, tag="wv")
            nc.sync.dma_start(wv_t[:], wv_d[:])

            for b in range(B):
                with tc.tile_pool(name=f"bt{b}", bufs=1) as btp:
                    # QKT[:, h, :] = Q^T head h ; QKT[:, 4+h, :] = K^T head h
                    QKT = btp.tile([128, 2 * HL, S], bf16, tag="qkt", name="qkt")
                    V = btp.tile([128, NTT, W], bf16, tag="v", name="v")
                    OTT = btp.tile([128, HL, S], bf16, tag="ott", name="ott")

                    # ---------------- phase 1: projections ----------------
                    with tc.tile_pool(name=f"p1x{b}", bufs=2) as xpool, \
                         tc.tile_pool(name=f"p1qk{b}", bufs=1, space="PSUM") as psqk, \
                         tc.tile_pool(name=f"p1v{b}", bufs=1, space="PSUM") as psv:
                        for c in range(NCH1):
                            xn = xpool.tile([128, NKB, CH1], bf16, tag="xn")
                            nc.sync.dma_start(xn[:], xh_d[b * NCH1 + c])
                            pqk = psqk.tile([128, 2 * HL, CH1], fp32, tag="pqk",
                                            name="pqk")
                            pv = psv.tile([128, 2, W], fp32, tag="pv", name="pv")
                            for kb in range(NKB):
                                for t in range(2 * HL):
                                    # two [128,256] tiles share one PSUM bank;
                                    # start clears the WHOLE bank's has_written
                                    # bits, so only the first matmul touching
                                    # each bank may carry start=True.
                                    nc.tensor.matmul(
                                        pqk[:, t, :],
                                        wqk_t[:, kb, t * 128:(t + 1) * 128],
                                        xn[:, kb, :],
                                        start=(kb == 0 and t % 2 == 0),
                                        stop=(kb == NKB - 1 and t % 2 == 1),
                                    )
                                for ts in range(2):
                                    nc.tensor.matmul(
                                        pv[:, ts, :],
                                        xn[:, kb, ts * 128:(ts + 1) * 128],
                                        wv_t[:, kb, :],
                                        start=(kb == 0), stop=(kb == NKB - 1),
                                    )
                            nc.scalar.copy(
                                QKT[:, :, c * CH1:(c + 1) * CH1], pqk[:])
                            nc.scalar.copy(V[:, 2 * c:2 * c + 2, :], pv[:])

                    # ---------------- phase 2: attention ----------------
                    # chunk = (head h, 512-query block sq); tails are emitted
                    # one chunk late so the PE never waits on the DVE tail.
                    with tc.tile_pool(name=f"a2e{b}", bufs=6) as ep, \
                         tc.tile_pool(name=f"a2s{b}", bufs=4) as esp, \
                         tc.tile_pool(name=f"a2r{b}", bufs=2) as rcp, \
                         tc.tile_pool(name=f"a2ps{b}", bufs=2, space="PSUM") as psS, \
                         tc.tile_pool(name=f"a2po{b}", bufs=2, space="PSUM") as psO, \
                         tc.tile_pool(name=f"a2pd{b}", bufs=1, space="PSUM") as psD, \
                         tc.tile_pool(name=f"a2pr{b}", bufs=1, space="PSUM") as psR:
                        state = {}

                        def emit_chunk(ci):
                            h, sq = ci
                            q0 = sq * 512
                            po = psO.tile([128, 512], fp32, tag="po", name="po")
                            esA = esp.tile([128, 512], bf16, tag="esA")
                            esB = esp.tile([128, 512], bf16, tag="esB")
                            dsum = psD.tile([1, 512], fp32, tag="dsum", name="dsum")
                            state[ci] = (po, esA, esB, dsum)
                            for g in range(8):
                                pS = psS.tile([128, 2, 512], fp32, tag="pS",
                                              name="pS")
                                for j in range(2):
                                    sk = 2 * g + j
                                    nc.tensor.matmul(
                                        pS[:, j, :],
                                        QKT[:, HL + h, sk * 128:(sk + 1) * 128],
                                        QKT[:, h, q0:q0 + 512],
                                        start=True, stop=True,
                                    )
                                eS = ep.tile([128, 2, 512], bf16, tag="eS")
                                nc.scalar.activation(eS[:], pS[:], EXP,
                                                     bias=zbias[:, 0:1])
                                for j in range(2):
                                    sk = 2 * g + j
                                    nc.tensor.matmul(
                                        po[:],
                                        V[:, sk, h * 128:(h + 1) * 128],
                                        eS[:, j, :],
                                        start=(sk == 0), stop=(sk == 15),
                                    )
                                    dst = esA if j == 0 else esB
                                    if g == 0:
                                        nc.vector.tensor_copy(dst[:], eS[:, j, :])
                                    else:
                                        nc.vector.tensor_add(dst[:], dst[:],
                                                             eS[:, j, :])
                            # partition-reduce the denominators on the PE
                            nc.vector.tensor_add(esA[:], esA[:], esB[:])
                            nc.tensor.matmul(dsum[:], ones_col[:], esA[:],
                                             start=True, stop=True)

                        def emit_tail(ci):
                            h, sq = ci
                            q0 = sq * 512
                            po, esA, esB, dsum = state.pop(ci)
                            ds = rcp.tile([1, 512], fp32, tag="ds")
                            nc.vector.tensor_copy(ds[:], dsum[:])
                            rr = rcp.tile([1, 512], fp32, tag="rr")
                            nc.vector.reciprocal_approx_fast(rr[:], ds[:])
                            rB = psR.tile([128, 512], fp32, tag="rB", name="rB")
                            nc.tensor.matmul(rB[:], ones_row[:], rr[:],
                                             start=True, stop=True)
                            rBs = rcp.tile([128, 512], fp32, tag="rBs")
                            nc.vector.tensor_copy(rBs[:], rB[:])
                            nc.vector.tensor_mul(
                                OTT[:, h, q0:q0 + 512], po[:], rBs[:])

                        prev = None
                        for ci in [(h, sq) for sq in range(4) for h in range(HL)]:
                            emit_chunk(ci)
                            if prev is not None:
                                emit_tail(prev)
                            prev = ci
                        emit_tail(prev)

                    # ---------------- phase 3: output projection ----------------
                    with tc.tile_pool(name=f"p3w{b}", bufs=1) as wop, \
                         tc.tile_pool(name=f"p3y{b}", bufs=4) as yp3, \
                         tc.tile_pool(name=f"p3ps{b}", bufs=4, space="PSUM") as psY:
                        wo_t = wop.tile([128, HL, E], bf16, tag="wo")
                        nc.sync.dma_start(wo_t[:], wo_d[:])
                        for nE in range(8):
                            for m in range(16):
                                py = psY.tile([128, 512], fp32, tag="py", name="py")
                                for kd in range(HL):
                                    nc.tensor.matmul(
                                        py[:],
                                        OTT[:, kd, m * 128:(m + 1) * 128],
                                        wo_t[:, kd, nE * 512:(nE + 1) * 512],
                                        start=(kd == 0), stop=(kd == HL - 1),
                                    )
                                yt = yp3.tile([128, 512], bf16, tag="yt")
                                if m % 2 == 0:
                                    nc.scalar.copy(yt[:], py[:])
                                else:
                                    nc.vector.tensor_copy(yt[:], py[:])
                                nc.sync.dma_start(
                                    y_d[b * S + m * 128: b * S + (m + 1) * 128,
                                        nE * 512:(nE + 1) * 512],
                                    yt[:],
                                )

    nc.compile()
    return nc


def _prep_inputs(x, freqs_cos, freqs_sin, wq, wk, wv, wo):
    x = np.asarray(x, np.float32)
    c = np.asarray(freqs_cos, np.float32)
    s = np.asarray(freqs_sin, np.float32)
    wq = np.asarray(wq, np.float32)
    wk = np.asarray(wk, np.float32)
    wv = np.asarray(wv, np.float32)
    wo = np.asarray(wo, np.float32)
    bf = ml_dtypes.bfloat16

    # x^T [E, T] -> per-chunk tiles [T/CH1, 128, NKB, CH1]
    xT = x.reshape(T, E).T.astype(bf)
    xh = np.ascontiguousarray(
        xT.reshape(NKB, 128, T // CH1, CH1).transpose(2, 1, 0, 3))

    def fold(w):
        wr = w.reshape(H, D // 2, 2, E)
        w0, w1 = wr[:, :, 0], wr[:, :, 1]
        r0 = c[:, :, None] * w0 - s[:, :, None] * w1
        r1 = s[:, :, None] * w0 + c[:, :, None] * w1
        return np.stack([r0, r1], axis=2).reshape(E, E)

    wq_r = fold(wq) * np.float32(D ** -0.5)
    wk_r = fold(wk)

    in_maps = []
    for cix in range(NCORES):
        sl = slice(cix * W, (cix + 1) * W)
        qk = np.concatenate([wq_r[sl].T, wk_r[sl].T], axis=1)   # [E, 2W]
        wqkh = np.ascontiguousarray(
            qk.astype(bf).reshape(NKB, 128, 2 * W).transpose(1, 0, 2))
        wvh = np.ascontiguousarray(
            wv[sl].T.astype(bf).reshape(NKB, 128, W).transpose(1, 0, 2))
        woh = np.ascontiguousarray(
            wo[:, sl].T.astype(bf).reshape(HL, 128, E).transpose(1, 0, 2))
        in_maps.append({"xh": xh, "wqk": wqkh, "wv": wvh, "wo": woh})
    return in_maps


def run(x, freqs_cos, freqs_sin, wq, wk, wv, wo, trace=False, tmpdir=None):
    from concourse.bass_utils import run_bass_kernel_spmd

    if "nc" not in _CACHE:
        _CACHE["nc"] = _build_nc()
    nc = _CACHE["nc"]
    in_maps = _prep_inputs(x, freqs_cos, freqs_sin, wq, wk, wv, wo)
    res = run_bass_kernel_spmd(
        nc, in_maps, list(range(NCORES)), trace=trace, tmpdir=tmpdir
    )
    y = np.asarray(res.results[0]["y"], np.float32)
    for r in res.results[1:]:
        y = y + np.asarray(r["y"], np.float32)
    return y.reshape(B, S, E), res


def kernel(x, start_pos=0, freqs_cos=None, freqs_sin=None,
           wq=None, wk=None, wv=None, wo=None):
    y, _ = run(x, freqs_cos, freqs_sin, wq, wk, wv, wo)
    return y


# revision 4
# speedup vs baseline: 1.3303x; 1.0380x over previous
"""Llama attention (B=2, S=2048, E=4096, H=32) on 8 trn2 NeuronCores.

Strategy (tensor-parallel over heads, 4 heads/core, all-bf16 datapath):
  - RoPE is position-independent here (cos/sin are [H, D/2]), so it is folded
    into wq/wk on the host; the 1/sqrt(D) scale is folded into wq too.
  - All matmul operands are bf16 (1 cycle/row on the PE, fast weight load),
    accumulation in fp32 PSUM.  The full E=4096 contraction of the Q/K/V
    projections accumulates in PSUM (32 chained matmuls per output tile), so
    no vector-engine adds are needed; a single scalar-engine copy (with bf16
    cast) evacuates each tile.
  - Scores are computed transposed (S^T = K^T-tile @ Q^T) and the attention
    output as O^T = V-tile @ P^T: zero on-device transposes.  exp() runs on
    the scalar engine over [128,1024] PSUM spans.  Softmax denominators:
    one wide bf16 DVE add per key-group -> gpsimd partition_all_reduce ->
    reciprocal_approx_fast -> one DVE multiply.  Softmax needs no
    max-subtraction (scores bounded ~ +-8; exp cannot overflow in fp32).
  - Per-core output is a partial Y (row-sharded wo) written bf16; the host
    sums the 8 partials in fp32.
"""

import sys

sys.path.insert(0, "/opt/trn_rl_repo")

import numpy as np
import ml_dtypes

B, S, E, H = 2, 2048, 4096, 32
D = 128            # head dim
NCORES = 8
HL = H // NCORES   # heads per core = 4
W = HL * D         # per-core projection width = 512
T = B * S          # 4096 tokens
NKB = 32           # 128-row contraction tiles over E
CH1 = 256          # phase-1 token chunk
NCH1 = S // CH1    # 8 chunks per batch
NTT = S // 128     # 16 token/key tiles per batch

_CACHE = {}


def _build_nc():
    import concourse.bass as bass  # noqa: F401
    import concourse.mybir as mybir
    import concourse.tile as tile
    from concourse import bacc
    from concourse.bass_isa import ReduceOp

    fp32 = mybir.dt.float32
    bf16 = mybir.dt.bfloat16
    EXP = mybir.ActivationFunctionType.Exp

    nc = bacc.Bacc("TRN2", target_bir_lowering=False, debug=False)

    xh_d = nc.dram_tensor("xh", [T // CH1, 128, NKB, CH1], bf16, kind="ExternalInput")
    wqk_d = nc.dram_tensor("wqk", [128, NKB, 2 * W], bf16, kind="ExternalInput")
    wv_d = nc.dram_tensor("wv", [128, NKB, W], bf16, kind="ExternalInput")
    wo_d = nc.dram_tensor("wo", [128, HL, E], bf16, kind="ExternalInput")
    y_d = nc.dram_tensor("y", [T, E], bf16, kind="ExternalOutput")

    with nc.allow_low_precision(reason="bf16 datapath; fp32 PSUM accumulation"), \
         tile.TileContext(nc) as tc:
        with tc.tile_pool(name="const", bufs=1) as constp, \
             tc.tile_pool(name="gw", bufs=1) as gwp, \
             tc.tile_pool(name="gwo", bufs=2) as wop:
            zbias = constp.tile([128, 1], fp32, tag="zbias")
            nc.vector.memset(zbias[:], 0.0)

            # resident weights (loaded in quarters, interleaved with the
            # first x chunk so the PE starts early)
            wqk_t = gwp.tile([128, NKB, 2 * W], bf16, tag="wqk")
            wv_t = gwp.tile([128, NKB, W], bf16, tag="wv")

            for b in range(B):
                with tc.tile_pool(name=f"bt{b}", bufs=1) as btp:
                    QKT = btp.tile([128, 2 * HL, S], bf16, tag="qkt", name="qkt")
                    V = btp.tile([128, NTT, W], bf16, tag="v", name="v")
                    OTT = btp.tile([128, HL, S], bf16, tag="ott", name="ott")

                    # ---------------- phase 1: projections ----------------
                    with tc.tile_pool(name=f"p1x{b}", bufs=2) as xpool, \
                         tc.tile_pool(name=f"p1qk{b}", bufs=1, space="PSUM") as psqk, \
                         tc.tile_pool(name=f"p1v{b}", bufs=2, space="PSUM") as psv:
                        for c in range(NCH1):
                            xn = xpool.tile([128, NKB, CH1], bf16, tag="xn")
                            nc.sync.dma_start(xn[:], xh_d[b * NCH1 + c])
                            if b == 0 and c == 0:
                                for q in range(4):
                                    sl = slice(q * 8, (q + 1) * 8)
                                    nc.sync.dma_start(wqk_t[:, sl, :],
                                                      wqk_d[:, sl, :])
                                    nc.sync.dma_start(wv_t[:, sl, :],
                                                      wv_d[:, sl, :])
                            pqk = psqk.tile([128, 2 * HL, CH1], fp32, tag="pqk",
                                            name="pqk")
                            pv = psv.tile([128, 2, W], fp32, tag="pv", name="pv")

                            def emit_v():
                                for kb in range(NKB):
                                    for ts in range(2):
                                        nc.tensor.matmul(
                                            pv[:, ts, :],
                                            xn[:, kb, ts * 128:(ts + 1) * 128],
                                            wv_t[:, kb, :],
                                            start=(kb == 0), stop=(kb == NKB - 1),
                                        )

                            def emit_qk():
                                for kb in range(NKB):
                                    for t in range(2 * HL):
                                        # two [128,256] tiles share one PSUM
                                        # bank; start clears the WHOLE bank's
                                        # has_written bits, so only the first
                                        # matmul touching each bank may carry
                                        # start=True.
                                        nc.tensor.matmul(
                                            pqk[:, t, :],
                                            wqk_t[:, kb, t * 128:(t + 1) * 128],
                                            xn[:, kb, :],
                                            start=(kb == 0 and t % 2 == 0),
                                            stop=(kb == NKB - 1 and t % 2 == 1),
                                        )

                            if c == 0:
                                emit_qk()
                                emit_v()
                            else:
                                emit_v()
                                emit_qk()
                            nc.scalar.copy(
                                QKT[:, :, c * CH1:(c + 1) * CH1], pqk[:])
                            nc.scalar.copy(V[:, 2 * c:2 * c + 2, :], pv[:])

                    # ---------------- phase 2: attention ----------------
                    # chunk = (head h, 512-query block sq); tails are emitted
                    # one chunk late so the PE never waits on the DVE tail.
                    with tc.tile_pool(name=f"a2e{b}", bufs=6) as ep, \
                         tc.tile_pool(name=f"a2s{b}", bufs=2) as esp, \
                         tc.tile_pool(name=f"a2r{b}", bufs=2) as rcp, \
                         tc.tile_pool(name=f"a2ps{b}", bufs=3, space="PSUM") as psS, \
                         tc.tile_pool(name=f"a2po{b}", bufs=2, space="PSUM") as psO:
                        # prefetch the first two wo slices for phase 3
                        wo_sl = [wop.tile([128, HL, 512], bf16, tag="wo",
                                          name=f"wo{b}_{i}") for i in range(8)]
                        nc.sync.dma_start(wo_sl[0][:], wo_d[:, :, 0:512])
                        nc.sync.dma_start(wo_sl[1][:], wo_d[:, :, 512:1024])

                        state = {}

                        def emit_chunk(ci):
                            h, sq = ci
                            q0 = sq * 512
                            po = psO.tile([128, 512], fp32, tag="po", name="po")
                            esAB = esp.tile([128, 2, 512], bf16, tag="esAB")
                            state[ci] = (po, esAB)
                            for g in range(8):
                                pS = psS.tile([128, 2, 512], fp32, tag="pS",
                                              name="pS")
                                for j in range(2):
                                    sk = 2 * g + j
                                    nc.tensor.matmul(
                                        pS[:, j, :],
                                        QKT[:, HL + h, sk * 128:(sk + 1) * 128],
                                        QKT[:, h, q0:q0 + 512],
                                        start=True, stop=True,
                                    )
                                eS = ep.tile([128, 2, 512], bf16, tag="eS")
                                nc.scalar.activation(eS[:], pS[:], EXP,
                                                     bias=zbias[:, 0:1])
                                for j in range(2):
                                    sk = 2 * g + j
                                    nc.tensor.matmul(
                                        po[:],
                                        V[:, sk, h * 128:(h + 1) * 128],
                                        eS[:, j, :],
                                        start=(sk == 0), stop=(sk == 15),
                                    )
                                if g == 0:
                                    nc.vector.tensor_copy(esAB[:], eS[:])
                                else:
                                    nc.vector.tensor_add(esAB[:], esAB[:], eS[:])

                        def emit_tail(ci):
                            h, sq = ci
                            q0 = sq * 512
                            po, esAB = state.pop(ci)
                            esum = rcp.tile([128, 512], fp32, tag="esum")
                            nc.vector.tensor_add(esum[:], esAB[:, 0, :],
                                                 esAB[:, 1, :])
                            denomB = rcp.tile([128, 512], fp32, tag="denomB")
                            nc.gpsimd.partition_all_reduce(
                                denomB[:], esum[:], 128, ReduceOp.add)
                            rr = rcp.tile([128, 512], fp32, tag="rr")
                            nc.vector.reciprocal_approx_fast(rr[:], denomB[:])
                            nc.vector.tensor_mul(
                                OTT[:, h, q0:q0 + 512], po[:], rr[:])

                        prev = None
                        for ci in [(h, sq) for sq in range(4) for h in range(HL)]:
                            emit_chunk(ci)
                            if prev is not None:
                                emit_tail(prev)
                            prev = ci
                        emit_tail(prev)

                    # ---------------- phase 3: output projection ----------------
                    with tc.tile_pool(name=f"p3y{b}", bufs=4) as yp3, \
                         tc.tile_pool(name=f"p3ps{b}", bufs=4, space="PSUM") as psY:
                        for nE in range(8):
                            wo_t = wo_sl[nE]
                            for m in range(16):
                                py = psY.tile([128, 512], fp32, tag="py", name="py")
                                for kd in range(HL):
                                    nc.tensor.matmul(
                                        py[:],
                                        OTT[:, kd, m * 128:(m + 1) * 128],
                                        wo_t[:, kd, :],
                                        start=(kd == 0), stop=(kd == HL - 1),
                                    )
                                yt = yp3.tile([128, 512], bf16, tag="yt")
                                if m % 2 == 0:
                                    nc.scalar.copy(yt[:], py[:])
                                else:
                                    nc.vector.tensor_copy(yt[:], py[:])
                                nc.sync.dma_start(
                                    y_d[b * S + m * 128: b * S + (m + 1) * 128,
                                        nE * 512:(nE + 1) * 512],
                                    yt[:],
                                )
                            if nE + 2 < 8:
                                nc.sync.dma_start(
                                    wo_sl[nE + 2][:],
                                    wo_d[:, :, (nE + 2) * 512:(nE + 3) * 512])

    nc.compile()
    return nc


def _prep_inputs(x, freqs_cos, freqs_sin, wq, wk, wv, wo):
    x = np.asarray(x, np.float32)
    c = np.asarray(freqs_cos, np.float32)
    s = np.asarray(freqs_sin, np.float32)
    wq = np.asarray(wq, np.float32)
    wk = np.asarray(wk, np.float32)
    wv = np.asarray(wv, np.float32)
    wo = np.asarray(wo, np.float32)
    bf = ml_dtypes.bfloat16

    xT = x.reshape(T, E).T.astype(bf)
    xh = np.ascontiguousarray(
        xT.reshape(NKB, 128, T // CH1, CH1).transpose(2, 1, 0, 3))

    def fold(w):
        wr = w.reshape(H, D // 2, 2, E)
        w0, w1 = wr[:, :, 0], wr[:, :, 1]
        r0 = c[:, :, None] * w0 - s[:, :, None] * w1
        r1 = s[:, :, None] * w0 + c[:, :, None] * w1
        return np.stack([r0, r1], axis=2).reshape(E, E)

    wq_r = fold(wq) * np.float32(D ** -0.5)
    wk_r = fold(wk)

    in_maps = []
    for cix in range(NCORES):
        sl = slice(cix * W, (cix + 1) * W)
        qk = np.concatenate([wq_r[sl].T, wk_r[sl].T], axis=1)   # [E, 2W]
        wqkh = np.ascontiguousarray(
            qk.astype(bf).reshape(NKB, 128, 2 * W).transpose(1, 0, 2))
        wvh = np.ascontiguousarray(
            wv[sl].T.astype(bf).reshape(NKB, 128, W).transpose(1, 0, 2))
        woh = np.ascontiguousarray(
            wo[:, sl].T.astype(bf).reshape(HL, 128, E).transpose(1, 0, 2))
        in_maps.append({"xh": xh, "wqk": wqkh, "wv": wvh, "wo": woh})
    return in_maps


def run(x, freqs_cos, freqs_sin, wq, wk, wv, wo, trace=False, tmpdir=None):
    from concourse.bass_utils import run_bass_kernel_spmd

    if "nc" not in _CACHE:
        _CACHE["nc"] = _build_nc()
    nc = _CACHE["nc"]
    in_maps = _prep_inputs(x, freqs_cos, freqs_sin, wq, wk, wv, wo)
    res = run_bass_kernel_spmd(
        nc, in_maps, list(range(NCORES)), trace=trace, tmpdir=tmpdir
    )
    y = np.asarray(res.results[0]["y"], np.float32)
    for r in res.results[1:]:
        y = y + np.asarray(r["y"], np.float32)
    return y.reshape(B, S, E), res


def kernel(x, start_pos=0, freqs_cos=None, freqs_sin=None,
           wq=None, wk=None, wv=None, wo=None):
    y, _ = run(x, freqs_cos, freqs_sin, wq, wk, wv, wo)
    return y
